# revision 17
# baseline (speedup 1.0000x reference)
"""Single-head causal attention (B=8, T=4096, EMB=1024, HEAD=64) on 8 trn2 cores.

Strategy: data-parallel over batch, one batch element per NeuronCore.

Per core (all matmuls in bf16, fp32 PSUM accumulation):
  1. x loaded as pre-transposed xT [1024, 4096] bf16, chunks split across the
     two HWDGE DMA queues (sync + scalar) to cut the load-phase from ~28us.
  2. KQ^T [128, 4096] (rows 0:64 = K^T, 64:128 = Q^T) in ONE pass over k
     (8 PSUM banks, k-outer j-inner).  At the last k step, each bank is
     copied out right after its final matmul, and SBUF->SBUF DMA copies
     build the swapped tile qk2 = [Q^T; K^T] in 1024-col slices so scores
     can start early.
  3. Scores use PE row-tiling (contraction d=64 only): even t-tiles run in
     array rows 0:63 (K^T lo x Q^T lo from kq/qk2), odd t-tiles in rows
     64:127 (K^T hi x Q^T hi) - two matmuls in flight -> ~2x scores rate.
  4. exp via ScalarE from PSUM (scale=1/8 folded), bf16 out, groups of <=2
     PSUM banks per instruction.  Causal: only tiles with t >= s computed;
     diagonal 128x128 block masked by a 0/1 multiply after exp.
  5. V projection (xT chunk stationary x Wv moving -> natural [t, 64])
     emitted in 2-row-block quarters interleaved into the attention loop so
     it fills PE slack instead of delaying the first exp.
  6. PV: P^T tile [128s, 128t] stationary, V-with-ones-column [128, 65]
     moving -> accumulates [O_unnorm | rowsum] in natural [t, 65] layout.
  7. out = O_unnorm * reciprocal(rowsum) (per-partition scalar), DMA out.
"""

from contextlib import ExitStack

import numpy as np
import ml_dtypes

B, T, EMB, HEAD = 8, 4096, 1024, 64
KCH = EMB // 128          # 8 contraction chunks
NTT = T // 512            # 8 t-tiles of 512
NTS = T // 128            # 32 t-subtiles / s-chunks of 128
BF16 = ml_dtypes.bfloat16

_CACHE = {}


def _build_program():
    import concourse.bacc as bacc
    import concourse.tile as tile
    from concourse import mybir

    fp32 = mybir.dt.float32
    bf16 = mybir.dt.bfloat16
    EXP = mybir.ActivationFunctionType.Exp

    nc = bacc.Bacc("TRN2", target_bir_lowering=False, debug=False)
    xt_ap = nc.dram_tensor("xt", [EMB, T], bf16, kind="ExternalInput").ap()
    # w2: [128, 1792] = per-partition-rearranged W (8 chunks x 192) ++ the
    # causal-mask factors Lc [128,128] and U [128,128] (see _host_prep)
    w_ap = nc.dram_tensor("w", [128, 1792], bf16, kind="ExternalInput").ap()
    o_ap = nc.dram_tensor("o", [T, HEAD], fp32, kind="ExternalOutput").ap()

    with tile.TileContext(nc) as tc:
        with (
            tc.tile_pool(name="consts", bufs=1) as consts,
            tc.tile_pool(name="outs", bufs=4) as outs,
        ):
            # ---------- phase 1: x chunks FIRST, alone on the sync HWDGE
            # queue.  One [128, 4096] instruction per chunk keeps all 16 SDMA
            # engines on one in-order ring: chunk k lands every ~2.5us
            # (~350 GB/s) and the KQ k-loop consumes chunks as they arrive.
            # DMA-completion semaphore lanes are a global pool of 8 recycled
            # in emission order, so the first 7 chunks + w2 get the 8 fresh
            # lanes (chunk 7 reuses chunk 0's lane - harmless, it is last
            # anyway).  Emitting w/mask on sync before the chunks would chain
            # every chunk behind a small-DMA completion.
            xstack = ExitStack()
            xp = xstack.enter_context(
                tc.tile_pool(name="xp", bufs=1, side="right")
            )
            xt_sb = xp.tile([128, KCH, T], bf16, tag="xt")
            for k in range(7):
                nc.sync.dma_start(
                    out=xt_sb[:, k, :], in_=xt_ap[k * 128:(k + 1) * 128, :]
                )
            w_sb = consts.tile([128, 1792], bf16, tag="w")
            nc.scalar.dma_start(out=w_sb, in_=w_ap)
            nc.sync.dma_start(out=xt_sb[:, 7, :], in_=xt_ap[7 * 128:, :])

            # warm the ACT exp table while DMAs run (first ACTIVATE of a set
            # pays ~2.7us of table-load otherwise right before the first real
            # exp on the critical path)
            warm = outs.tile([128, 1], fp32, tag="warm")
            nc.scalar.activation(warm, w_sb[:, 0:1], EXP)
            # V with ones column: [128, s-chunk, 65]; col 64 preset to 1.0
            vt_sb = consts.tile([128, NTS, 65], bf16, tag="vt")
            nc.gpsimd.memset(vt_sb, 1.0)

            lc_ap = w_sb[:, 1536:1664]   # Lc[p, s] = -240 * (s > p)
            u_ap = w_sb[:, 1664:1792]    # U[p, t]  = (p >= t)
            kq_sb = consts.tile([128, T], bf16, tag="kq")    # [K^T; Q^T]
            qk2_sb = consts.tile([128, T], bf16, tag="qk2")  # [Q^T; K^T]

            with tc.tile_pool(name="ps_kq", bufs=1, space="PSUM") as ps_kq:
                pkq = []
                for j in range(NTT):
                    pkq_j = ps_kq.tile([128, 512], fp32, tag=f"kq{j}")
                    pkq.append(pkq_j)
                for k in range(KCH):
                    for j in range(NTT):
                        nc.tensor.matmul(
                            pkq[j],
                            w_sb[:, k * 192:k * 192 + 128],
                            xt_sb[:, k, j * 512:(j + 1) * 512],
                            start=(k == 0),
                            stop=(k == KCH - 1),
                            skip_group_check=True,
                        )
                        if k == KCH - 1:
                            # bank j is final: drain it while the remaining
                            # last-k matmuls run
                            nc.vector.tensor_copy(
                                kq_sb[:, j * 512:(j + 1) * 512], pkq[j]
                            )
                            if j % 2 == 1:
                                # swapped-layout slices for row-tiled scores
                                lo = (j - 1) * 512
                                hi = (j + 1) * 512
                                nc.gpsimd.dma_start(
                                    out=qk2_sb[0:64, lo:hi],
                                    in_=kq_sb[64:128, lo:hi],
                                )
                                nc.scalar.dma_start(
                                    out=qk2_sb[64:128, lo:hi],
                                    in_=kq_sb[0:64, lo:hi],
                                )
            del pkq

            # ---------- phase 2: attention (+ interleaved V projection) ----
            phase2 = ExitStack()
            ptA = phase2.enter_context(tc.tile_pool(name="ptA", bufs=1))
            ps_s = phase2.enter_context(tc.tile_pool(name="ps_s", bufs=2, space="PSUM"))
            ps_v = phase2.enter_context(tc.tile_pool(name="ps_v", bufs=2, space="PSUM"))
            ps_o = phase2.enter_context(tc.tile_pool(name="ps_o", bufs=2, space="PSUM"))
            pt = [None] * NTS
            ptB_pool = [None]

            vstate = {"blk": None}

            def emit_vquarter(q):
                """V projection for rows i = 2q, 2q+1 (k-inner, 1 bank/block)."""
                if q % 4 == 0:
                    vstate["blk"] = ps_v.tile(
                        [128, 8, 64], fp32, tag="vblk", name=f"vblk{q // 4}"
                    )
                blk = vstate["blk"]
                for i in (2 * q, 2 * q + 1):
                    for k in range(KCH):
                        nc.tensor.matmul(
                            blk[:, i % 8, :],
                            xt_sb[:, k, i * 128:(i + 1) * 128],
                            w_sb[:, k * 192 + 128:(k + 1) * 192],
                            start=(k == 0 and i % 8 == 0),
                            stop=(k == KCH - 1),
                            skip_group_check=True,
                        )
                if q % 4 == 3:
                    # one strided copy per 8-row block: fewer PE<->DVE PSUM
                    # bank serializations than per-row copies
                    b = q // 4
                    nc.vector.tensor_copy(
                        vt_sb[:, 8 * b:8 * b + 8, 0:64], blk
                    )

            def score_groups(a):
                """[(jstart, gsize), ...] groups of <=2 t-tiles for s-chunk a."""
                j0 = a // 4
                groups = []
                j = j0
                while j < NTT:
                    g = min(2, NTT - j)
                    groups.append((j, g))
                    j += g
                return groups

            def emit_scores(a):
                tiles = []
                for (jstart, g) in score_groups(a):
                    psg = ps_s.tile([128, 512 * g], fp32, tag="sg",
                                    padded_shape=[128, 1024],
                                    name=f"sg{a}_{jstart}")
                    for idx in range(g):
                        j = jstart + idx
                        has_diag = (j == a // 4)
                        if j % 2 == 0:
                            # PE rows 0:63 - K^T lo stationary, Q^T lo moving
                            nc.tensor.matmul(
                                psg[:, idx * 512:(idx + 1) * 512],
                                kq_sb[0:64, a * 128:(a + 1) * 128],
                                qk2_sb[0:64, j * 512:(j + 1) * 512],
                                start=True,
                                stop=not has_diag,
                                skip_group_check=True,
                            )
                        else:
                            # PE rows 64:127 - K^T hi stationary, Q^T hi moving
                            nc.tensor.matmul(
                                psg[:, idx * 512:(idx + 1) * 512],
                                qk2_sb[64:128, a * 128:(a + 1) * 128],
                                kq_sb[64:128, j * 512:(j + 1) * 512],
                                start=True,
                                stop=not has_diag,
                                skip_group_check=True,
                            )
                        if has_diag:
                            # causal mask: S[s,t] += -240*max(0, s-t) on the
                            # diagonal 128x128 block, so exp() underflows to
                            # exactly 0 where s > t (no DVE mask multiply)
                            d0 = idx * 512 + 128 * a - 512 * jstart
                            nc.tensor.matmul(
                                psg[:, d0:d0 + 128],
                                lc_ap,
                                u_ap,
                                start=False,
                                stop=True,
                                skip_group_check=True,
                            )
                    tiles.append((jstart, g, psg))
                return tiles

            def emit_exp(a, tiles):
                pool = ptA if a < 16 else ptB_pool[0]
                pt[a] = pool.tile([128, T - 128 * a], bf16, tag=f"pt{a}",
                                  name=f"pt{a}")
                for (jstart, g, psg) in tiles:
                    skip = max(0, 128 * a - 512 * jstart)
                    out_lo = 512 * jstart + skip - 128 * a
                    out_hi = 512 * (jstart + g) - 128 * a
                    nc.scalar.activation(
                        pt[a][:, out_lo:out_hi],
                        psg[:, skip:512 * g],
                        EXP,
                        scale=0.125,
                    )

            def emit_pv(i):
                po = ps_o.tile([128, 65], fp32, tag="o", name=f"po{i}",
                               padded_shape=[128, 512])
                for aa in range(i + 1):
                    nc.tensor.matmul(
                        po,
                        pt[aa][:, 128 * (i - aa):128 * (i - aa) + 128],
                        vt_sb[:, aa, :],
                        start=(aa == 0),
                        stop=(aa == i),
                    )
                dr = outs.tile([128, 1], fp32, tag="dr")
                nc.vector.reciprocal(dr, po[:, 64:65])
                o_sb = outs.tile([128, 64], fp32, tag="o_sb")
                nc.vector.tensor_scalar_mul(o_sb, po[:, 0:64], dr)
                nc.sync.dma_start(out=o_ap[i * 128:(i + 1) * 128, :], in_=o_sb)

            # one V quarter fills the PE gap between the last KQ matmul and
            # the first scores matmul (which waits on the qk2 copies); two
            # more follow scores(0) so they run while ACT drains exp(0)
            emit_vquarter(0)
            tiles = emit_scores(0)
            emit_vquarter(1)
            emit_vquarter(2)
            # software-pipelined: while ACT(a) drains, PE runs S(a+1); PV for
            # t-tile i fires once chunk i is exp'd; V quarters fill PE slack.
            for a in range(NTS):
                if a == 16:
                    # all V work done (last xt use was quarter 15 at a=12):
                    # free the x tile and place the remaining P tiles there
                    xstack.close()
                    ptB_pool[0] = phase2.enter_context(
                        tc.tile_pool(name="ptB", bufs=1)
                    )
                emit_exp(a, tiles)
                if a + 1 < NTS:
                    tiles = emit_scores(a + 1)
                if a >= 1:
                    emit_pv(a - 1)
                if a + 3 <= 15:
                    emit_vquarter(a + 3)
            emit_pv(NTS - 1)
            phase2.close()

    nc.compile()
    return nc


def _get_nc():
    if "nc" not in _CACHE:
        _CACHE["nc"] = _build_program()
    return _CACHE["nc"]


def _host_prep(x, W):
    """-> (xt [B, EMB, T] bf16, w2 [128, 1792] bf16)."""
    x = np.asarray(x, dtype=np.float32)
    W = np.asarray(W, dtype=np.float32)
    assert x.shape == (B, T, EMB) and W.shape == (EMB, 3 * HEAD)

    xt = np.ascontiguousarray(x.transpose(0, 2, 1)).astype(BF16)  # [B, EMB, T]
    # w2[p, k*192:(k+1)*192] = W[k*128+p, :]; then Lc and U mask factors
    w2 = np.zeros((128, 1792), np.float32)
    w2[:, 0:1536] = W.reshape(KCH, 128, 192).transpose(1, 0, 2).reshape(128, 1536)
    p = np.arange(128)
    w2[:, 1536:1664] = np.where(p[None, :] > p[:, None], -240.0, 0.0)  # Lc[r,s]
    w2[:, 1664:1792] = np.where(p[:, None] >= p[None, :], 1.0, 0.0)    # U[r,t]
    return xt, w2.astype(BF16)


def kernel(x, W):
    from concourse.bass_utils import run_bass_kernel_spmd

    xt, w2 = _host_prep(x, W)
    nc = _get_nc()
    in_maps = [{"xt": xt[b], "w": w2} for b in range(B)]
    res = run_bass_kernel_spmd(nc, in_maps, list(range(B)))
    return np.stack([res.results[b]["o"] for b in range(B)]).astype(np.float32)


# revision 25
# speedup vs baseline: 1.3387x; 1.3387x over previous
"""Single-head causal attention (B=8, T=4096, EMB=1024, HEAD=64) on 8 trn2 cores.

Strategy: data-parallel over batch, one batch element per NeuronCore.

Per core (all matmuls in bf16, fp32 PSUM accumulation):
  1. x loaded as pre-transposed xT [1024, 4096] bf16, chunks split across the
     two HWDGE DMA queues (sync + scalar) to cut the load-phase from ~28us.
  2. KQ^T [128, 4096] (rows 0:64 = K^T, 64:128 = Q^T) in ONE pass over k
     (8 PSUM banks, k-outer j-inner).  At the last k step, each bank is
     copied out right after its final matmul, and SBUF->SBUF DMA copies
     build the swapped tile qk2 = [Q^T; K^T] in 1024-col slices so scores
     can start early.
  3. Scores use PE row-tiling (contraction d=64 only): even t-tiles run in
     array rows 0:63 (K^T lo x Q^T lo from kq/qk2), odd t-tiles in rows
     64:127 (K^T hi x Q^T hi) - two matmuls in flight -> ~2x scores rate.
  4. exp via ScalarE from PSUM (scale=1/8 folded), bf16 out, groups of <=2
     PSUM banks per instruction.  Causal: only tiles with t >= s computed;
     diagonal 128x128 block masked by a 0/1 multiply after exp.
  5. V projection (xT chunk stationary x Wv moving -> natural [t, 64])
     emitted in 2-row-block quarters interleaved into the attention loop so
     it fills PE slack instead of delaying the first exp.
  6. PV: P^T tile [128s, 128t] stationary, V-with-ones-column [128, 65]
     moving -> accumulates [O_unnorm | rowsum] in natural [t, 65] layout.
  7. out = O_unnorm * reciprocal(rowsum) (per-partition scalar), DMA out.
"""

from contextlib import ExitStack

import numpy as np
import ml_dtypes

B, T, EMB, HEAD = 8, 4096, 1024, 64
KCH = EMB // 128          # 8 contraction chunks
NTT = T // 512            # 8 t-tiles of 512
NTS = T // 128            # 32 t-subtiles / s-chunks of 128
BF16 = ml_dtypes.bfloat16

_CACHE = {}


def _build_program():
    import concourse.bacc as bacc
    import concourse.tile as tile
    from concourse import mybir

    fp32 = mybir.dt.float32
    bf16 = mybir.dt.bfloat16
    EXP = mybir.ActivationFunctionType.Exp

    nc = bacc.Bacc("TRN2", target_bir_lowering=False, debug=False)
    xt_ap = nc.dram_tensor("xt", [EMB, T], bf16, kind="ExternalInput").ap()
    # w2: [128, 1664] = per-partition-rearranged W (8 chunks x 192) ++ the
    # 0/1 upper-triangular diag mask [128,128] (see _host_prep)
    w_ap = nc.dram_tensor("w", [128, 1664], bf16, kind="ExternalInput").ap()
    o_ap = nc.dram_tensor("o", [T, HEAD], fp32, kind="ExternalOutput").ap()

    with tile.TileContext(nc) as tc:
        with (
            tc.tile_pool(name="consts", bufs=1) as consts,
            tc.tile_pool(name="outs", bufs=4) as outs,
        ):
            # ---------- phase 1: x chunks FIRST, alone on the sync HWDGE
            # queue.  One [128, 4096] instruction per chunk keeps all 16 SDMA
            # engines on one in-order ring: chunk k lands every ~2.5us
            # (~350 GB/s) and the KQ k-loop consumes chunks as they arrive.
            # DMA-completion semaphore lanes are a global pool of 8 recycled
            # in emission order, so the first 7 chunks + w2 get the 8 fresh
            # lanes (chunk 7 reuses chunk 0's lane - harmless, it is last
            # anyway).  Emitting w/mask on sync before the chunks would chain
            # every chunk behind a small-DMA completion.
            xstack = ExitStack()
            xp = xstack.enter_context(
                tc.tile_pool(name="xp", bufs=1, side="right")
            )
            xt_sb = xp.tile([128, KCH, T], bf16, tag="xt")
            for k in range(7):
                nc.sync.dma_start(
                    out=xt_sb[:, k, :], in_=xt_ap[k * 128:(k + 1) * 128, :]
                )
            w_sb = consts.tile([128, 1664], bf16, tag="w")
            nc.scalar.dma_start(out=w_sb, in_=w_ap)
            nc.sync.dma_start(out=xt_sb[:, 7, :], in_=xt_ap[7 * 128:, :])

            # warm the ACT exp table while DMAs run (first ACTIVATE of a set
            # pays ~2.7us of table-load otherwise right before the first real
            # exp on the critical path)
            warm = outs.tile([128, 1], fp32, tag="warm")
            nc.scalar.activation(warm, w_sb[:, 0:1], EXP)
            # V with ones column: [128, s-chunk, 65]; col 64 preset to 1.0
            vt_sb = consts.tile([128, NTS, 65], bf16, tag="vt")
            nc.gpsimd.memset(vt_sb, 1.0)

            mask_ap = w_sb[:, 1536:1664]   # mask[s, t] = (s <= t)
            kq_sb = consts.tile([128, T], bf16, tag="kq")    # [K^T; Q^T]
            qk2_sb = consts.tile([128, T], bf16, tag="qk2")  # [Q^T; K^T]

            with tc.tile_pool(name="ps_kq", bufs=1, space="PSUM") as ps_kq:
                pkq = []
                for j in range(NTT):
                    pkq_j = ps_kq.tile([128, 512], fp32, tag=f"kq{j}")
                    pkq.append(pkq_j)
                for k in range(KCH):
                    for j in range(NTT):
                        nc.tensor.matmul(
                            pkq[j],
                            w_sb[:, k * 192:k * 192 + 128],
                            xt_sb[:, k, j * 512:(j + 1) * 512],
                            start=(k == 0),
                            stop=(k == KCH - 1),
                            skip_group_check=True,
                        )
                        if k == KCH - 1:
                            # bank j is final: drain it while the remaining
                            # last-k matmuls run
                            nc.vector.tensor_copy(
                                kq_sb[:, j * 512:(j + 1) * 512], pkq[j]
                            )
                            if j % 2 == 1:
                                # swapped-layout slices for row-tiled scores
                                lo = (j - 1) * 512
                                hi = (j + 1) * 512
                                nc.gpsimd.dma_start(
                                    out=qk2_sb[0:64, lo:hi],
                                    in_=kq_sb[64:128, lo:hi],
                                )
                                nc.scalar.dma_start(
                                    out=qk2_sb[64:128, lo:hi],
                                    in_=kq_sb[0:64, lo:hi],
                                )
            del pkq

            # ---------- phase 2: attention (+ interleaved V projection) ----
            phase2 = ExitStack()
            ptA = phase2.enter_context(tc.tile_pool(name="ptA", bufs=1))
            ps_s = phase2.enter_context(tc.tile_pool(name="ps_s", bufs=2, space="PSUM"))
            ps_v = phase2.enter_context(tc.tile_pool(name="ps_v", bufs=2, space="PSUM"))
            ps_o = phase2.enter_context(tc.tile_pool(name="ps_o", bufs=2, space="PSUM"))
            pt = [None] * NTS
            ptB_pool = [None]

            vstate = {"blk": None}

            def emit_vquarter(q):
                """V projection for rows i = 2q, 2q+1 (k-inner, 1 bank/block)."""
                if q % 4 == 0:
                    vstate["blk"] = ps_v.tile(
                        [128, 8, 64], fp32, tag="vblk", name=f"vblk{q // 4}"
                    )
                blk = vstate["blk"]
                for i in (2 * q, 2 * q + 1):
                    for k in range(KCH):
                        nc.tensor.matmul(
                            blk[:, i % 8, :],
                            xt_sb[:, k, i * 128:(i + 1) * 128],
                            w_sb[:, k * 192 + 128:(k + 1) * 192],
                            start=(k == 0 and i % 8 == 0),
                            stop=(k == KCH - 1),
                            skip_group_check=True,
                        )
                if q % 4 == 3:
                    # one strided copy per 8-row block: fewer PE<->DVE PSUM
                    # bank serializations than per-row copies
                    b = q // 4
                    nc.vector.tensor_copy(
                        vt_sb[:, 8 * b:8 * b + 8, 0:64], blk
                    )

            def score_groups(a):
                """[(jstart, gsize), ...] groups of <=2 t-tiles for s-chunk a."""
                j0 = a // 4
                groups = []
                j = j0
                while j < NTT:
                    g = min(2, NTT - j)
                    groups.append((j, g))
                    j += g
                return groups

            def emit_scores(a):
                tiles = []
                for (jstart, g) in score_groups(a):
                    psg = ps_s.tile([128, 512 * g], fp32, tag="sg",
                                    padded_shape=[128, 1024],
                                    name=f"sg{a}_{jstart}")
                    for idx in range(g):
                        j = jstart + idx
                        if j % 2 == 0:
                            # PE rows 0:63 - K^T lo stationary, Q^T lo moving
                            nc.tensor.matmul(
                                psg[:, idx * 512:(idx + 1) * 512],
                                kq_sb[0:64, a * 128:(a + 1) * 128],
                                qk2_sb[0:64, j * 512:(j + 1) * 512],
                                start=True,
                                stop=True,
                            )
                        else:
                            # PE rows 64:127 - K^T hi stationary, Q^T hi moving
                            nc.tensor.matmul(
                                psg[:, idx * 512:(idx + 1) * 512],
                                qk2_sb[64:128, a * 128:(a + 1) * 128],
                                kq_sb[64:128, j * 512:(j + 1) * 512],
                                start=True,
                                stop=True,
                            )
                    tiles.append((jstart, g, psg))
                return tiles

            def emit_exp(a, tiles):
                pool = ptA if a < 16 else ptB_pool[0]
                pt[a] = pool.tile([128, T - 128 * a], bf16, tag=f"pt{a}",
                                  name=f"pt{a}")
                for (jstart, g, psg) in tiles:
                    skip = max(0, 128 * a - 512 * jstart)
                    out_lo = 512 * jstart + skip - 128 * a
                    out_hi = 512 * (jstart + g) - 128 * a
                    nc.scalar.activation(
                        pt[a][:, out_lo:out_hi],
                        psg[:, skip:512 * g],
                        EXP,
                        scale=0.125,
                    )
                # mask the diagonal 128x128 block (zero where s > t)
                nc.vector.tensor_mul(pt[a][:, 0:128], pt[a][:, 0:128], mask_ap)

            def emit_pv(i):
                po = ps_o.tile([128, 65], fp32, tag="o", name=f"po{i}",
                               padded_shape=[128, 512])
                for aa in range(i + 1):
                    nc.tensor.matmul(
                        po,
                        pt[aa][:, 128 * (i - aa):128 * (i - aa) + 128],
                        vt_sb[:, aa, :],
                        start=(aa == 0),
                        stop=(aa == i),
                    )
                dr = outs.tile([128, 1], fp32, tag="dr")
                nc.vector.reciprocal(dr, po[:, 64:65])
                o_sb = outs.tile([128, 64], fp32, tag="o_sb")
                nc.vector.tensor_scalar_mul(o_sb, po[:, 0:64], dr)
                nc.sync.dma_start(out=o_ap[i * 128:(i + 1) * 128, :], in_=o_sb)

            # scores(0) goes first (its qk2 wait is ~2us of PE idle, under
            # the HAM re-throttle window); V quarters follow so they run on
            # PE while ACT drains exp(0)
            tiles = emit_scores(0)
            emit_vquarter(0)
            emit_vquarter(1)
            emit_vquarter(2)
            # software-pipelined: while ACT(a) drains, PE runs S(a+1); PV for
            # t-tile i fires once chunk i is exp'd; V quarters fill PE slack.
            for a in range(NTS):
                if a == 16:
                    # all V work done (last xt use was quarter 15 at a=12):
                    # free the x tile and place the remaining P tiles there
                    xstack.close()
                    ptB_pool[0] = phase2.enter_context(
                        tc.tile_pool(name="ptB", bufs=1)
                    )
                emit_exp(a, tiles)
                if a + 1 < NTS:
                    tiles = emit_scores(a + 1)
                if a >= 1:
                    emit_pv(a - 1)
                if a + 3 <= 15:
                    emit_vquarter(a + 3)
            emit_pv(NTS - 1)
            phase2.close()

    nc.compile()
    return nc


def _get_nc():
    if "nc" not in _CACHE:
        _CACHE["nc"] = _build_program()
    return _CACHE["nc"]


def _host_prep(x, W):
    """-> (xt [B, EMB, T] bf16, w2 [128, 1664] bf16)."""
    x = np.asarray(x, dtype=np.float32)
    W = np.asarray(W, dtype=np.float32)
    assert x.shape == (B, T, EMB) and W.shape == (EMB, 3 * HEAD)

    xt = np.ascontiguousarray(x.transpose(0, 2, 1)).astype(BF16)  # [B, EMB, T]
    # w2[p, k*192:(k+1)*192] = W[k*128+p, :]; then the 0/1 diag-block mask
    w2 = np.zeros((128, 1664), np.float32)
    w2[:, 0:1536] = W.reshape(KCH, 128, 192).transpose(1, 0, 2).reshape(128, 1536)
    w2[:, 1536:1664] = np.triu(np.ones((128, 128), np.float32))
    return xt, w2.astype(BF16)


def kernel(x, W):
    from concourse.bass_utils import run_bass_kernel_spmd

    xt, w2 = _host_prep(x, W)
    nc = _get_nc()
    in_maps = [{"xt": xt[b], "w": w2} for b in range(B)]
    res = run_bass_kernel_spmd(nc, in_maps, list(range(B)))
    return np.stack([res.results[b]["o"] for b in range(B)]).astype(np.float32)


# revision 34
# speedup vs baseline: 1.3917x; 1.0396x over previous
"""Single-head causal attention (B=8, T=4096, EMB=1024, HEAD=64) on 8 trn2 cores.

Strategy: data-parallel over batch, one batch element per NeuronCore.

Per core (all matmuls in bf16, fp32 PSUM accumulation):
  1. x loaded as pre-transposed xT [1024, 4096] bf16, chunks split across the
     two HWDGE DMA queues (sync + scalar) to cut the load-phase from ~28us.
  2. KQ^T [128, 4096] (rows 0:64 = K^T, 64:128 = Q^T) in ONE pass over k
     (8 PSUM banks, k-outer j-inner).  At the last k step, each bank is
     copied out right after its final matmul, and SBUF->SBUF DMA copies
     build the swapped tile qk2 = [Q^T; K^T] in 1024-col slices so scores
     can start early.
  3. Scores use PE row-tiling (contraction d=64 only): even t-tiles run in
     array rows 0:63 (K^T lo x Q^T lo from kq/qk2), odd t-tiles in rows
     64:127 (K^T hi x Q^T hi) - two matmuls in flight -> ~2x scores rate.
  4. exp via ScalarE from PSUM (scale=1/8 folded), bf16 out, groups of <=2
     PSUM banks per instruction.  Causal: only tiles with t >= s computed;
     diagonal 128x128 block masked by a 0/1 multiply after exp.
  5. V projection (xT chunk stationary x Wv moving -> natural [t, 64])
     emitted in 2-row-block quarters interleaved into the attention loop so
     it fills PE slack instead of delaying the first exp.
  6. PV: P^T tile [128s, 128t] stationary, V-with-ones-column [128, 65]
     moving -> accumulates [O_unnorm | rowsum] in natural [t, 65] layout.
  7. out = O_unnorm * reciprocal(rowsum) (per-partition scalar), DMA out.
"""

from contextlib import ExitStack

import numpy as np
import ml_dtypes

B, T, EMB, HEAD = 8, 4096, 1024, 64
KCH = EMB // 128          # 8 contraction chunks
NTT = T // 512            # 8 t-tiles of 512
NTS = T // 128            # 32 t-subtiles / s-chunks of 128
BF16 = ml_dtypes.bfloat16

_CACHE = {}


def _build_program():
    import concourse.bacc as bacc
    import concourse.tile as tile
    from concourse import mybir

    fp32 = mybir.dt.float32
    bf16 = mybir.dt.bfloat16
    EXP = mybir.ActivationFunctionType.Exp

    nc = bacc.Bacc("TRN2", target_bir_lowering=False, debug=False)
    xt_ap = nc.dram_tensor("xt", [EMB, T], bf16, kind="ExternalInput").ap()
    # w2: [128, 1664] = per-partition-rearranged W (8 chunks x 192) ++ the
    # 0/1 upper-triangular diag mask [128,128] (see _host_prep)
    w_ap = nc.dram_tensor("w", [128, 1664], bf16, kind="ExternalInput").ap()
    o_ap = nc.dram_tensor("o", [T, HEAD], fp32, kind="ExternalOutput").ap()

    with tile.TileContext(nc) as tc:
        with (
            tc.tile_pool(name="consts", bufs=1) as consts,
            tc.tile_pool(name="outs", bufs=4) as outs,
        ):
            # ---------- phase 1: x chunks FIRST, alone on the sync HWDGE
            # queue.  One [128, 4096] instruction per chunk keeps all 16 SDMA
            # engines on one in-order ring: chunk k lands every ~2.5us
            # (~350 GB/s) and the KQ k-loop consumes chunks as they arrive.
            # DMA-completion semaphore lanes are a global pool of 8 recycled
            # in emission order, so the first 7 chunks + w2 get the 8 fresh
            # lanes (chunk 7 reuses chunk 0's lane - harmless, it is last
            # anyway).  Emitting w/mask on sync before the chunks would chain
            # every chunk behind a small-DMA completion.
            xstack = ExitStack()
            xp = xstack.enter_context(
                tc.tile_pool(name="xp", bufs=1, side="right")
            )
            xt_sb = xp.tile([128, KCH, T], bf16, tag="xt")
            for k in range(7):
                nc.sync.dma_start(
                    out=xt_sb[:, k, :], in_=xt_ap[k * 128:(k + 1) * 128, :]
                )
            w_sb = consts.tile([128, 1664], bf16, tag="w")
            nc.scalar.dma_start(out=w_sb, in_=w_ap)
            nc.sync.dma_start(out=xt_sb[:, 7, :], in_=xt_ap[7 * 128:, :])

            # warm the ACT exp table while DMAs run (first ACTIVATE of a set
            # pays ~2.7us of table-load otherwise right before the first real
            # exp on the critical path)
            warm = outs.tile([128, 1], fp32, tag="warm")
            nc.scalar.activation(warm, w_sb[:, 0:1], EXP)
            # V with ones column: [128, s-chunk, 65]; col 64 preset to 1.0
            vt_sb = consts.tile([128, NTS, 65], bf16, tag="vt")
            nc.gpsimd.memset(vt_sb, 1.0)
            zeros_sb = consts.tile([128, 65], fp32, tag="zeros")
            nc.gpsimd.memset(zeros_sb, 0.0)

            mask_ap = w_sb[:, 1536:1664]   # mask[s, t] = (s <= t)
            kq_sb = consts.tile([128, T], bf16, tag="kq")    # [K^T; Q^T]
            qk2_sb = consts.tile([128, T], bf16, tag="qk2")  # [Q^T; K^T]

            with tc.tile_pool(name="ps_kq", bufs=1, space="PSUM") as ps_kq:
                pkq = []
                for j in range(NTT):
                    pkq_j = ps_kq.tile([128, 512], fp32, tag=f"kq{j}")
                    pkq.append(pkq_j)
                # dummy matmuls on garbage SBUF while the x chunks stream in:
                # ~7us of sustained PE activity flips the HAM clock-gate to
                # 8/8 before the first real KQ matmul, instead of running the
                # first ~12us of KQ at the 1.2 GHz cold clock.  Results land
                # in bank 0 and are cleared by KQ's first start=True.
                for _ in range(26):
                    nc.tensor.matmul(
                        pkq[0],
                        kq_sb[:, 0:128],
                        kq_sb[:, 0:512],
                        start=True,
                        stop=True,
                        skip_group_check=True,
                    )
                for k in range(KCH):
                    for j in range(NTT):
                        nc.tensor.matmul(
                            pkq[j],
                            w_sb[:, k * 192:k * 192 + 128],
                            xt_sb[:, k, j * 512:(j + 1) * 512],
                            start=(k == 0),
                            stop=(k == KCH - 1),
                            skip_group_check=True,
                        )
                        if k == KCH - 1:
                            # bank j is final: drain it while the remaining
                            # last-k matmuls run
                            nc.vector.tensor_copy(
                                kq_sb[:, j * 512:(j + 1) * 512], pkq[j]
                            )
                            if j % 2 == 1:
                                # swapped-layout slices for row-tiled scores
                                lo = (j - 1) * 512
                                hi = (j + 1) * 512
                                nc.gpsimd.dma_start(
                                    out=qk2_sb[0:64, lo:hi],
                                    in_=kq_sb[64:128, lo:hi],
                                )
                                nc.scalar.dma_start(
                                    out=qk2_sb[64:128, lo:hi],
                                    in_=kq_sb[0:64, lo:hi],
                                )
            del pkq

            # ---------- phase 2: attention (+ interleaved V projection) ----
            phase2 = ExitStack()
            ptA = phase2.enter_context(tc.tile_pool(name="ptA", bufs=1))
            ps_s = phase2.enter_context(tc.tile_pool(name="ps_s", bufs=2, space="PSUM"))
            ps_o = phase2.enter_context(tc.tile_pool(name="ps_o", bufs=2, space="PSUM"))
            ps_v = tc.alloc_tile_pool(name="ps_v", bufs=2, space="PSUM",
                                      side="right")
            pt = [None] * NTS
            ptB_pool = [None]
            # partial-PV slot banks (allocated at a=13 when ps_v releases):
            # slot for row-block i lives in pp[i%2][:, (i//2)%4, :]
            pp = [None, None]

            vstate = {"blk": None}

            def emit_vquarter(q):
                """V projection for rows i = 2q, 2q+1 (k-inner, 1 bank/block)."""
                if q % 4 == 0:
                    vstate["blk"] = ps_v.tile(
                        [128, 8, 64], fp32, tag="vblk", name=f"vblk{q // 4}"
                    )
                blk = vstate["blk"]
                for i in (2 * q, 2 * q + 1):
                    for k in range(KCH):
                        nc.tensor.matmul(
                            blk[:, i % 8, :],
                            xt_sb[:, k, i * 128:(i + 1) * 128],
                            w_sb[:, k * 192 + 128:(k + 1) * 192],
                            start=(k == 0 and i % 8 == 0),
                            stop=(k == KCH - 1),
                            skip_group_check=True,
                        )
                if q % 4 == 3:
                    # one strided copy per 8-row block: fewer PE<->DVE PSUM
                    # bank serializations than per-row copies
                    b = q // 4
                    nc.vector.tensor_copy(
                        vt_sb[:, 8 * b:8 * b + 8, 0:64], blk
                    )

            def score_groups(a):
                """[(jstart, gsize), ...] groups of <=2 t-tiles for s-chunk a."""
                j0 = a // 4
                groups = []
                j = j0
                while j < NTT:
                    g = min(2, NTT - j)
                    groups.append((j, g))
                    j += g
                return groups

            def emit_scores(a):
                tiles = []
                for (jstart, g) in score_groups(a):
                    psg = ps_s.tile([128, 512 * g], fp32, tag="sg",
                                    padded_shape=[128, 1024],
                                    name=f"sg{a}_{jstart}")
                    for idx in range(g):
                        j = jstart + idx
                        if j % 2 == 0:
                            # PE rows 0:63 - K^T lo stationary, Q^T lo moving
                            nc.tensor.matmul(
                                psg[:, idx * 512:(idx + 1) * 512],
                                kq_sb[0:64, a * 128:(a + 1) * 128],
                                qk2_sb[0:64, j * 512:(j + 1) * 512],
                                start=True,
                                stop=True,
                            )
                        else:
                            # PE rows 64:127 - K^T hi stationary, Q^T hi moving
                            nc.tensor.matmul(
                                psg[:, idx * 512:(idx + 1) * 512],
                                qk2_sb[64:128, a * 128:(a + 1) * 128],
                                kq_sb[64:128, j * 512:(j + 1) * 512],
                                start=True,
                                stop=True,
                            )
                    tiles.append((jstart, g, psg))
                return tiles

            def emit_exp(a, tiles):
                pool = ptA if a < 16 else ptB_pool[0]
                pt[a] = pool.tile([128, T - 128 * a], bf16, tag=f"pt{a}",
                                  name=f"pt{a}")
                for (jstart, g, psg) in tiles:
                    skip = max(0, 128 * a - 512 * jstart)
                    out_lo = 512 * jstart + skip - 128 * a
                    out_hi = 512 * (jstart + g) - 128 * a
                    nc.scalar.activation(
                        pt[a][:, out_lo:out_hi],
                        psg[:, skip:512 * g],
                        EXP,
                        scale=0.125,
                    )
                # mask the diagonal 128x128 block (zero where s > t)
                nc.vector.tensor_mul(pt[a][:, 0:128], pt[a][:, 0:128], mask_ap)

            def norm_out(i, po):
                dr = outs.tile([128, 1], fp32, tag="dr", name=f"dr{i}")
                nc.vector.reciprocal(dr, po[:, 64:65])
                o_sb = outs.tile([128, 64], fp32, tag="o_sb", name=f"osb{i}")
                nc.vector.tensor_scalar_mul(o_sb, po[:, 0:64], dr)
                nc.sync.dma_start(out=o_ap[i * 128:(i + 1) * 128, :], in_=o_sb)

            def emit_pv(i):
                """Full PV chain for row-block i (used for i <= 18)."""
                po = ps_o.tile([128, 65], fp32, tag="o", name=f"po{i}",
                               padded_shape=[128, 512])
                for aa in range(i + 1):
                    nc.tensor.matmul(
                        po,
                        pt[aa][:, 128 * (i - aa):128 * (i - aa) + 128],
                        vt_sb[:, aa, :],
                        start=(aa == 0),
                        stop=(aa == i),
                    )
                norm_out(i, po)

            PLAG = 6  # partial born at chunk i-PLAG covers aa < i-PLAG

            def pv_slot(i):
                return pp[i % 2][:, (i // 2) % 4, 0:65]

            def emit_pv_birth(i):
                """Start row-block i's PV accumulation early: zero the slot
                (DVE write; matmuls then add via stale has_written bits or
                overwrite the zeros on a fresh bank - both correct) and chain
                all already-exp'd chunks aa < i-PLAG."""
                slot = pv_slot(i)
                nc.vector.tensor_copy(slot, zeros_sb)
                for aa in range(i - PLAG):
                    nc.tensor.matmul(
                        slot,
                        pt[aa][:, 128 * (i - aa):128 * (i - aa) + 128],
                        vt_sb[:, aa, :],
                        start=False,
                        stop=False,
                        skip_group_check=True,
                    )

            def emit_pv_final(i):
                """Finish row-block i (i >= 19): last PLAG+1 chunks + norm."""
                slot = pv_slot(i)
                for aa in range(i - PLAG, i + 1):
                    nc.tensor.matmul(
                        slot,
                        pt[aa][:, 128 * (i - aa):128 * (i - aa) + 128],
                        vt_sb[:, aa, :],
                        start=False,
                        stop=(aa == i),
                        skip_group_check=True,
                    )
                norm_out(i, slot)

            # scores(0) goes first (its qk2 wait is ~2us of PE idle, under
            # the HAM re-throttle window); V quarters follow so they run on
            # PE while ACT drains exp(0)
            tiles = emit_scores(0)
            emit_vquarter(0)
            emit_vquarter(1)
            emit_vquarter(2)
            # software-pipelined: while ACT(a) drains, PE runs S(a+1); PV for
            # t-tile i fires once chunk i is exp'd; V quarters fill PE slack.
            for a in range(NTS):
                if a == 13:
                    # V projection fully emitted (last quarter at a=12):
                    # swap its 2 PSUM banks for the partial-PV slot banks
                    ps_v.release()
                    ps_pp = phase2.enter_context(
                        tc.tile_pool(name="ps_pp", bufs=1, space="PSUM",
                                     side="right")
                    )
                    pp[0] = ps_pp.tile([128, 4, 128], fp32, tag="ppA",
                                       name="ppA")
                    pp[1] = ps_pp.tile([128, 4, 128], fp32, tag="ppB",
                                       name="ppB")
                if a == 16:
                    # all V work done (last xt use was quarter 15 at a=12):
                    # free the x tile and place the remaining P tiles there
                    xstack.close()
                    ptB_pool[0] = phase2.enter_context(
                        tc.tile_pool(name="ptB", bufs=1)
                    )
                emit_exp(a, tiles)
                if a + 1 < NTS:
                    tiles = emit_scores(a + 1)
                if a >= 1:
                    i = a - 1
                    if i <= 12 + PLAG:
                        emit_pv(i)
                    else:
                        emit_pv_final(i)
                if 13 <= a <= NTS - 1 - PLAG:
                    emit_pv_birth(a + PLAG)
                if a + 3 <= 15:
                    emit_vquarter(a + 3)
            emit_pv_final(NTS - 1)
            phase2.close()

    nc.compile()
    return nc


def _get_nc():
    if "nc" not in _CACHE:
        _CACHE["nc"] = _build_program()
    return _CACHE["nc"]


def _host_prep(x, W):
    """-> (xt [B, EMB, T] bf16, w2 [128, 1664] bf16)."""
    x = np.asarray(x, dtype=np.float32)
    W = np.asarray(W, dtype=np.float32)
    assert x.shape == (B, T, EMB) and W.shape == (EMB, 3 * HEAD)

    xt = np.ascontiguousarray(x.transpose(0, 2, 1)).astype(BF16)  # [B, EMB, T]
    # w2[p, k*192:(k+1)*192] = W[k*128+p, :]; then the 0/1 diag-block mask
    w2 = np.zeros((128, 1664), np.float32)
    w2[:, 0:1536] = W.reshape(KCH, 128, 192).transpose(1, 0, 2).reshape(128, 1536)
    w2[:, 1536:1664] = np.triu(np.ones((128, 128), np.float32))
    return xt, w2.astype(BF16)


def kernel(x, W):
    from concourse.bass_utils import run_bass_kernel_spmd

    xt, w2 = _host_prep(x, W)
    nc = _get_nc()
    in_maps = [{"xt": xt[b], "w": w2} for b in range(B)]
    res = run_bass_kernel_spmd(nc, in_maps, list(range(B)))
    return np.stack([res.results[b]["o"] for b in range(B)]).astype(np.float32)
